# revision 23
# baseline (speedup 1.0000x reference)
"""Trainium2 Bass kernel for nn_Net_20512763805724 (dense_mlp, 3-layer SLP net).

Math (per layer, input p [B,L], weight w [O,L]):
    wb = sign(w)  (w>=0 -> +1 else -1)
    e  = 2p-1 ; d = 4p(1-p)
    out = (sum(d+e^2) + (e@wb.T)^2 - (e^2)@(wb^2).T) / L^2
Since d+e^2 == 1 exactly and wb^2 == 1:
    out[b,o] = (C[b] + s1[b,o]^2) / L^2
    s1 = e@wb.T,   C[b] = sum_f (1 - e^2) = L - sum_f e^2

Layer 1 therefore only needs e and e^2:
    psA = wb1T.T @ eT          (s1, 64 outputs per batch tile)
    psB = (-s)*ones.T @ (e^2)T (-s*sum e^2; +s*L added as a constant)
    p2  = Square(psA/L) + s*L + psB
The host ships eT = (2x-1) pre-tiled/transposed in bf16 (pure affine fold
into the layout/dtype transform); all reductions, squares and the
nonlinear q=p(1-p) chains stay on device.

Sharding: pure data parallel over batch, 8 cores x 8192 rows, features on
SBUF partitions, batch on the free dim.  All weight prep (binarize,
column sums -> biases, replication) is host-side.  Output is outT
[4, 8192] per core, un-transposed on the host.

Per pair of 512-batch tiles, instruction-interleaved so the two tiles'
matmuls run CONCURRENTLY in different PE column groups (tile_position
(0,0)/(0,64)); e^2 is split DVE (chunks 0..3, tensor_tensor mult, 2x
bf16 mode) / ACT (chunks 4..5, Square).  Layers 2/3 stream p and p^2
(computed by one DVE mult) through +/-s ones matmuls instead of
materializing q = p-p^2.  A 3-stage software pipeline (head / C+combine
one pair later / quad combine one further iteration later) keeps every
op's dependencies at least one pipeline stage old.  Output DMAs and
weight loads ride the gpsimd queue; x rides sync.
"""

import sys

if "/opt/trn_rl_repo" not in sys.path:
    sys.path.insert(0, "/opt/trn_rl_repo")

import ml_dtypes
import numpy as np

BF16 = ml_dtypes.bfloat16

B = 65536
IN_DIM = 768
NCORES = 8
BC = B // NCORES            # 8192 rows per core
TILE = 512                  # batch tile (max fp32 PSUM free dim)
NT = BC // TILE             # 16 tiles per core
NPAIR = NT // 2
NCHUNK = IN_DIM // 128      # 6 feature chunks for layer 1
L1, O1 = 768, 64
L2, O2 = 64, 32
L3, O3 = 32, 4
DVE_SPLIT = 1792            # e^2 cols 0..1791 on DVE (3.5 chunks); rest ACT
ACT_SPLIT = NCHUNK * TILE - DVE_SPLIT
XBUFS = 5                   # x pair-tile prefetch depth (pairs)
# B-matmul segments: (source, src_lo, src_hi, out_lo, out_hi); sources are
# the DVE e^2 tile (chunk-major cols 0..1791) and ACT e^2 tile (1792..3071).
B_SEGS = [
    ("d", 0, 512, 0, 512),
    ("d", 512, 1024, 0, 512),
    ("d", 1024, 1536, 0, 512),
    ("d", 1536, 1792, 0, 256),
    ("a", 0, 256, 256, 512),
    ("a", 256, 768, 0, 512),
    ("a", 768, 1280, 0, 512),
]
# C-term scale: the ones lhsT holds -s in bf16, so use the bf16-rounded
# value as the effective scale for the +s*L1 constant too.
S1_EFF = float(np.float32(BF16(1.0 / (L1 * L1))))
K1 = S1_EFF * L1            # +s*L1 constant  (C = L - sum e^2)
S2 = 4.0 / (L2 * L2)        # 2^-10, exact in bf16
S3 = 4.0 / (L3 * L3)        # 2^-8, exact in bf16

_CACHE = {}


def _build(reps=1, mm_order="interleaved"):
    import contextlib

    import concourse.bacc as bacc
    import concourse.mybir as mybir
    import concourse.tile as tile

    f32 = mybir.dt.float32
    bf16 = mybir.dt.bfloat16
    AOP = mybir.AluOpType
    Square = mybir.ActivationFunctionType.Square

    nc = bacc.Bacc(None, target_bir_lowering=False)

    xt = nc.declare_dram_parameter("xt", [NPAIR, 128, 2, NCHUNK * TILE],
                                   bf16, isOutput=False)
    w1d = nc.declare_dram_parameter("w1d", [128, NCHUNK, O1], bf16,
                                    isOutput=False)
    w2d = nc.declare_dram_parameter("w2d", [128, O2], bf16, isOutput=False)
    w3d = nc.declare_dram_parameter("w3d", [128, L3], bf16, isOutput=False)
    b2d = nc.declare_dram_parameter("b2d", [128, 1], f32, isOutput=False)
    b3d = nc.declare_dram_parameter("b3d", [128, 1], f32, isOutput=False)
    outt = nc.declare_dram_parameter("outt", [O3, BC], f32, isOutput=True)

    NQUAD = NPAIR // 2

    with tile.TileContext(nc) as tc:
        with (
            tc.tile_pool(name="const", bufs=1) as cpool,
            tc.tile_pool(name="xp", bufs=XBUFS) as xpool,
            tc.tile_pool(name="sed", bufs=3) as sedpool,
            tc.tile_pool(name="sea", bufs=3) as seapool,
            tc.tile_pool(name="sb", bufs=2) as spool,
            tc.tile_pool(name="psA", bufs=2, space="PSUM") as pA,
            tc.tile_pool(name="psB", bufs=2, space="PSUM") as pB,
            tc.tile_pool(name="psS", bufs=1, space="PSUM") as pS,
        ):
            # ------------- constants (host-precomputed weights) -------------
            lhsT1 = cpool.tile([128, NCHUNK, O1], bf16)
            nc.gpsimd.dma_start(out=lhsT1, in_=w1d[:])
            lhsT2 = cpool.tile([128, O2], bf16)
            nc.gpsimd.dma_start(out=lhsT2, in_=w2d[:])
            lhsT3 = cpool.tile([128, L3], bf16)
            nc.gpsimd.dma_start(out=lhsT3, in_=w3d[:])
            bias2 = cpool.tile([128, 1], f32)
            nc.gpsimd.dma_start(out=bias2, in_=b2d[:])
            bias3 = cpool.tile([128, 1], f32)
            nc.gpsimd.dma_start(out=bias3, in_=b3d[:])

            onesN1 = cpool.tile([128, O1], bf16)
            nc.vector.memset(onesN1, -S1_EFF)
            onesP2 = cpool.tile([128, O2], bf16)
            nc.vector.memset(onesP2, S2)
            onesN2 = cpool.tile([128, O2], bf16)
            nc.vector.memset(onesN2, -S2)
            onesP3 = cpool.tile([128, L3], bf16)
            nc.vector.memset(onesP3, S3)
            onesN3 = cpool.tile([128, L3], bf16)
            nc.vector.memset(onesN3, -S3)
            zbias = cpool.tile([128, 1], f32)
            nc.vector.memset(zbias, 0.0)

            # PE warmup: ~4.5us of dummy matmuls while the first x pair is
            # still in flight, so the HAM clock-gate reaches 2.4 GHz before
            # real matmuls start (otherwise the first ~3.4us run at 1.2 GHz).
            wtile = cpool.tile([128, TILE], bf16)
            nc.vector.memset(wtile, 0.0)
            psW = pS.tile([128, TILE], f32, tag="A3", name="warm")
            for w in range(20):
                nc.tensor.matmul(
                    psW[0:O1, :], wtile[:, 0:O1], wtile,
                    tile_position=(0, 0),
                )

            # ------------- pipeline stages -------------
            state = {}   # pair -> (se_d, se_a, psA1, psB1)
            cstate = {}  # pair -> (p2p, sq2p) ; quad -> psum/combine tiles

            def head(pr):
                """DMA + s1-matmuls + e^2 for pair pr."""
                x_p = xpool.tile([128, 2, NCHUNK * TILE], bf16, tag="x",
                                 name=f"x_{pr}")
                nc.sync.dma_start(out=x_p, in_=xt[pr])
                psA1 = pA.tile([128, TILE], f32, tag="A1", name=f"psA1_{pr}")
                psB1 = pB.tile([128, TILE], f32, tag="B1", name=f"psB1_{pr}")
                # interleaved s1 matmuls: the two tiles alternate column
                # groups every instruction -> concurrent in the PE array
                if mm_order == "interleaved":
                    a_iter = [(c, k) for c in range(NCHUNK) for k in range(2)]
                else:
                    a_iter = [(c, k) for k in range(2) for c in range(NCHUNK)]
                for c, k in a_iter:
                    nc.tensor.matmul(
                        psA1[O1 * k : O1 * (k + 1), :],
                        lhsT1[:, c, :],
                        x_p[:, k, c * TILE : (c + 1) * TILE],
                        start=(c == 0),
                        stop=(c == NCHUNK - 1),
                        tile_position=(0, O1 * k),
                    )
                # e^2: per-tile 2D ops (3D APs measure ~25% slower)
                se_d = sedpool.tile([128, 2, DVE_SPLIT], bf16, tag="sed",
                                    name=f"sed_{pr}")
                se_a = seapool.tile([128, 2, ACT_SPLIT], bf16, tag="sea",
                                    name=f"sea_{pr}")
                for k in range(2):
                    nc.vector.tensor_mul(
                        se_d[:, k, :],
                        x_p[:, k, 0:DVE_SPLIT],
                        x_p[:, k, 0:DVE_SPLIT],
                    )
                for k in range(2):
                    nc.scalar.activation(
                        se_a[:, k, :], x_p[:, k, DVE_SPLIT:], Square,
                        bias=zbias, scale=1.0,
                    )
                state[pr] = (se_d, se_a, psA1, psB1)

            def tailB(pr):
                """C-term matmuls + layer-1 combine + layer-2 prep, pair pr.

                Emits: PE B-MMs, ACT t2p, DVE p2p + sq2p.  (The layer-2
                matmuls are emitted separately in tailL2 so the PE can run
                the quad's layer-3 matmuls first.)
                """
                se_d, se_a, psA1, psB1 = state.pop(pr)
                # C-term: psB1 = (-s)*ones.T @ e^2, interleaved tiles
                nseg = len(B_SEGS)
                if mm_order == "interleaved":
                    b_iter = [(s, k) for s in range(nseg) for k in range(2)]
                else:
                    b_iter = [(s, k) for k in range(2) for s in range(nseg)]
                for s, k in b_iter:
                    which, lo, hi, olo, ohi = B_SEGS[s]
                    tile_src = se_d if which == "d" else se_a
                    nc.tensor.matmul(
                        psB1[O1 * k : O1 * k + O1, olo:ohi],
                        onesN1,
                        tile_src[:, k, lo:hi],
                        start=(s == 0),
                        stop=(s == nseg - 1),
                        tile_position=(0, O1 * k),
                    )
                t2p = spool.tile([128, TILE], f32, tag="t2", name=f"t2_{pr}")
                nc.scalar.activation(
                    t2p, psA1, Square, bias=zbias, scale=1.0 / L1
                )
                p2p = spool.tile([128, TILE], bf16, tag="p2", name=f"p2_{pr}")
                nc.vector.scalar_tensor_tensor(
                    p2p, t2p, K1, psB1, AOP.add, AOP.add
                )
                sq2p = spool.tile([128, TILE], bf16, tag="sq2",
                                  name=f"sq2_{pr}")
                nc.vector.tensor_mul(sq2p, p2p, p2p)
                cstate[pr] = (p2p, sq2p)

            def tailL2(pr):
                """Layer-2 matmuls for pair pr (p2 and p2^2 streams)."""
                p2p, sq2p = cstate.pop(pr)
                qj = (2 * pr + 1) % 4
                if qj == 1:
                    psA2 = pA.tile([128, TILE], f32, tag="A2", bufs=1,
                                   name=f"psA2_{pr}")
                    psB2 = pB.tile([128, TILE], f32, tag="B2", bufs=1,
                                   name=f"psB2_{pr}")
                    cstate[("q", pr // 2)] = (psA2, psB2)
                else:
                    psA2, psB2 = cstate[("q", pr // 2)]
                # each tile of the pair produces its own 32-row block:
                # psA2 gets one K=64 matmul per block; psB2 accumulates the
                # p2-stream and the (-)p2^2-stream per block.  half-
                # interleaved so the two blocks run concurrently.
                halves = ((0, qj - 1), (1, qj))
                for half, tq in halves:
                    hs = slice(O1 * half, O1 * (half + 1))
                    nc.tensor.matmul(
                        psA2[O2 * tq : O2 * (tq + 1), :],
                        lhsT2[hs, :],
                        p2p[hs, :],
                        tile_position=(O1 * half, O2 * tq),
                    )
                for half, tq in halves:
                    hs = slice(O1 * half, O1 * (half + 1))
                    nc.tensor.matmul(
                        psB2[O2 * tq : O2 * (tq + 1), :],
                        onesP2[hs, :],
                        p2p[hs, :],
                        start=True,
                        stop=False,
                        tile_position=(O1 * half, O2 * tq),
                    )
                for half, tq in halves:
                    hs = slice(O1 * half, O1 * (half + 1))
                    nc.tensor.matmul(
                        psB2[O2 * tq : O2 * (tq + 1), :],
                        onesN2[hs, :],
                        sq2p[hs, :],
                        start=False,
                        stop=True,
                        tile_position=(O1 * half, O2 * tq),
                    )

            def quad_pre(Q):
                """Layer-2 combine, ACT part (earliest, old deps)."""
                psA2, psB2 = cstate.pop(("q", Q))
                tq2 = spool.tile([128, TILE], f32, tag="tq2", name=f"tq2_{Q}")
                nc.scalar.activation(
                    tq2, psA2, Square, bias=bias2, scale=2.0 / L2
                )
                cstate[("qc", Q)] = (psB2, tq2)

            def quad_dve(Q):
                """Layer-2 combine, DVE part -> p3, p3^2."""
                psB2, tq2 = cstate.pop(("qc", Q))
                p3q = spool.tile([128, TILE], bf16, tag="p3", name=f"p3_{Q}")
                nc.vector.scalar_tensor_tensor(
                    p3q, tq2, 0.0, psB2, AOP.add, AOP.add
                )
                sq3q = spool.tile([128, TILE], bf16, tag="sq3",
                                  name=f"sq3_{Q}")
                nc.vector.tensor_mul(sq3q, p3q, p3q)
                cstate[("q3", Q)] = (p3q, sq3q)

            def quad_l3(Q):
                """Layer-3 matmuls + combine + output DMA for quad Q."""
                p3q, sq3q = cstate.pop(("q3", Q))
                psA3 = pS.tile([128, TILE], f32, tag="A3", name=f"psA3_{Q}")
                psB3 = pS.tile([128, TILE], f32, tag="B3", name=f"psB3_{Q}")
                # j-interleaved: the 4 diagonal 32x32 positions run
                # concurrently in the PE array (3 slot-times per quad)
                for j in range(4):
                    js = slice(32 * j, 32 * j + L3)
                    nc.tensor.matmul(
                        psA3[32 * j : 32 * j + L3, :],
                        lhsT3[js, :],
                        p3q[js, :],
                        tile_position=(32 * j, 32 * j),
                    )
                for j in range(4):
                    js = slice(32 * j, 32 * j + L3)
                    nc.tensor.matmul(
                        psB3[32 * j : 32 * j + L3, :],
                        onesP3[js, :],
                        p3q[js, :],
                        start=True,
                        stop=False,
                        tile_position=(32 * j, 32 * j),
                    )
                for j in range(4):
                    js = slice(32 * j, 32 * j + L3)
                    nc.tensor.matmul(
                        psB3[32 * j : 32 * j + L3, :],
                        onesN3[js, :],
                        sq3q[js, :],
                        start=False,
                        stop=True,
                        tile_position=(32 * j, 32 * j),
                    )
                t3q = spool.tile([128, TILE], f32, tag="t3", name=f"t3_{Q}")
                nc.scalar.activation(
                    t3q, psA3, Square, bias=bias3, scale=2.0 / L3
                )
                outq = spool.tile([128, TILE], f32, tag="outq",
                                  name=f"outq_{Q}")
                nc.vector.scalar_tensor_tensor(
                    outq, t3q, 0.0, psB3, AOP.add, AOP.add
                )
                for j in range(4):
                    tt = 4 * Q + j
                    nc.gpsimd.dma_start(
                        out=outt[:, tt * TILE : (tt + 1) * TILE],
                        in_=outq[32 * j : 32 * j + O3, :],
                    )

            loop_cm = (
                tc.For_i(0, reps, 1) if reps > 1 else contextlib.nullcontext()
            )
            with loop_cm:
                for it in range(NPAIR + 3):
                    Q = (it - 4) // 2
                    do_quad = it >= 4 and (it - 4) % 2 == 0 and Q < NQUAD
                    if do_quad:
                        quad_pre(Q)
                    if it < NPAIR:
                        head(it)
                    if do_quad:
                        quad_dve(Q)
                    if 1 <= it <= NPAIR:
                        tailB(it - 1)
                    if do_quad:
                        quad_l3(Q)
                    if 2 <= it <= NPAIR + 1:
                        tailL2(it - 2)

    nc.compile()
    return nc


def _get_nc(reps=1, mm_order="interleaved"):
    key = ("nc", reps, mm_order)
    if key not in _CACHE:
        _CACHE[key] = _build(reps, mm_order)
    return _CACHE[key]


def _make_in_maps(x, w1, w2, w3):
    x = np.asarray(x, dtype=np.float32)
    w1 = np.asarray(w1, dtype=np.float32)
    w2 = np.asarray(w2, dtype=np.float32)
    w3 = np.asarray(w3, dtype=np.float32)

    # binarized weights and their column sums (all tiny -> host)
    wb1 = np.where(w1 >= 0, 1.0, -1.0).astype(np.float32)   # [64, 768]
    wb2 = np.where(w2 >= 0, 1.0, -1.0).astype(np.float32)   # [32, 64]
    wb3 = np.where(w3 >= 0, 1.0, -1.0).astype(np.float32)   # [4, 32]

    # lhsT1[p, c, o] = wb1[o, c*128+p]
    w1d = np.ascontiguousarray(
        wb1.T.reshape(NCHUNK, 128, O1).transpose(1, 0, 2)
    ).astype(BF16)
    # lhsT2: [128, 32], rows 0..63 = wb2.T, rows 64..127 = copy
    w2d = np.ascontiguousarray(np.tile(wb2.T, (2, 1))).astype(BF16)
    # lhsT3: [128, 32], wb3.T replicated 4x in cols 0..3, zeros in 4..31
    # (zero columns keep the padded psum rows written so every PSUM read
    #  touches initialized memory)
    w3d = np.zeros((128, L3), np.float32)
    w3d[:, :O3] = np.tile(wb3.T, (4, 1))
    w3d = np.ascontiguousarray(w3d).astype(BF16)

    c2 = wb2.sum(axis=1)   # [32]
    c3 = wb3.sum(axis=1)   # [4]
    b2d = np.ascontiguousarray(
        np.tile(-c2 / L2, 4).reshape(128, 1)
    ).astype(np.float32)
    b3 = np.zeros((4, 32), np.float32)
    b3[:, :O3] = -c3 / L3
    b3d = np.ascontiguousarray(b3.reshape(128, 1))

    xs = x.reshape(NCORES, NT, TILE, NCHUNK, 128)
    # eT = 2x-1, [core][pair, partition(f%128), tile-in-pair,
    #             chunk(f//128)*TILE + batch-in-tile], bf16
    et = (xs.transpose(0, 1, 4, 3, 2) * np.float32(2.0) - np.float32(1.0))
    etiled = np.ascontiguousarray(et.astype(BF16)).reshape(
        NCORES, NPAIR, 2, 128, NCHUNK * TILE
    ).transpose(0, 1, 3, 2, 4)
    etiled = np.ascontiguousarray(etiled)
    return [
        {
            "xt": etiled[i],
            "w1d": w1d,
            "w2d": w2d,
            "w3d": w3d,
            "b2d": b2d,
            "b3d": b3d,
        }
        for i in range(NCORES)
    ]


def kernel(x, w1, w2, w3):
    from concourse.bass_utils import run_bass_kernel_spmd

    nc = _get_nc()
    in_maps = _make_in_maps(x, w1, w2, w3)
    res = run_bass_kernel_spmd(nc, in_maps, core_ids=list(range(NCORES)))
    return np.concatenate(
        [res.results[i]["outt"].T for i in range(NCORES)], axis=0
    ).astype(np.float32)


def bench(x, w1, w2, w3, iters=20, reps=1, cores=NCORES):
    """Time device execution with a persistent jit and device-resident
    inputs (excludes host<->device transfer and compile).  Returns
    (output, per_call_seconds_list).  NOTE: per-call wall time under axon
    is dominated by a fixed ~80ms relay dispatch latency; use the NTFF
    profile (run_bass_kernel_spmd(trace=True)) for true HW exec time."""
    import time

    import jax
    from jax.sharding import Mesh, NamedSharding, PartitionSpec
    from jax.experimental.shard_map import shard_map

    import concourse.mybir as mybir
    from concourse import bass2jax
    from concourse.bass2jax import _bass_exec_p, install_neuronx_cc_hook

    nc = _get_nc(reps)
    install_neuronx_cc_hook()
    in_maps = _make_in_maps(x, w1, w2, w3)

    partition_name = (
        nc.partition_id_tensor.name if nc.partition_id_tensor else None
    )
    in_names, out_names, out_avals, zero_outs = [], [], [], []
    for alloc in nc.m.functions[0].allocations:
        if not isinstance(alloc, mybir.MemoryLocationSet):
            continue
        name = alloc.memorylocations[0].name
        if alloc.kind == "ExternalInput":
            if name != partition_name:
                in_names.append(name)
        elif alloc.kind == "ExternalOutput":
            out_names.append(name)
            shape = tuple(alloc.tensor_shape)
            dtype = mybir.dt.np(alloc.dtype)
            out_avals.append(jax.core.ShapedArray(shape, dtype))
            zero_outs.append(np.zeros(shape, dtype))
    n_params = len(in_names)
    in_names = in_names + out_names
    if partition_name is not None:
        in_names = in_names + [partition_name]

    def _body(*args):
        operands = list(args)
        if partition_name is not None:
            operands.append(bass2jax.partition_id_tensor())
        outs = _bass_exec_p.bind(
            *operands,
            out_avals=tuple(out_avals),
            in_names=tuple(in_names),
            out_names=tuple(out_names),
            lowering_input_output_aliases=(),
            sim_require_finite=True,
            sim_require_nnan=True,
            nc=nc,
        )
        return tuple(outs)

    devices = jax.devices()[:cores]
    mesh = Mesh(np.asarray(devices), ("core",))
    in_specs = (PartitionSpec("core"),) * (n_params + len(out_names))
    out_specs = (PartitionSpec("core"),) * len(out_names)
    fn = jax.jit(
        shard_map(_body, mesh=mesh, in_specs=in_specs, out_specs=out_specs,
                  check_rep=False),
        keep_unused=True,
    )
    sh = NamedSharding(mesh, PartitionSpec("core"))
    dev_in = [
        jax.device_put(
            np.concatenate([in_maps[c][nm] for c in range(cores)], axis=0), sh
        )
        for nm in in_names[:n_params]
    ]
    dev_zero = [
        jax.device_put(
            np.zeros((cores * z.shape[0], *z.shape[1:]), z.dtype), sh
        )
        for z in zero_outs
    ]
    out = fn(*dev_in, *dev_zero)
    jax.block_until_ready(out)
    times = []
    for _ in range(iters):
        t0 = time.perf_counter()
        out = fn(*dev_in, *dev_zero)
        jax.block_until_ready(out)
        times.append(time.perf_counter() - t0)
    out_np = np.asarray(out[0]).reshape(cores, *out_avals[0].shape)
    result = np.concatenate([out_np[c].T for c in range(cores)], axis=0)
    return result.astype(np.float32), times


# revision 25
# speedup vs baseline: 1.0659x; 1.0659x over previous
"""Trainium2 Bass kernel for nn_Net_20512763805724 (dense_mlp, 3-layer SLP net).

Math (per layer, input p [B,L], weight w [O,L]):
    wb = sign(w)  (w>=0 -> +1 else -1)
    e  = 2p-1 ; d = 4p(1-p)
    out = (sum(d+e^2) + (e@wb.T)^2 - (e^2)@(wb^2).T) / L^2
Since d+e^2 == 1 exactly and wb^2 == 1:
    out[b,o] = (C[b] + s1[b,o]^2) / L^2
    s1 = e@wb.T,   C[b] = sum_f (1 - e^2) = L - sum_f e^2

Layer 1 therefore only needs e and e^2:
    psA = wb1T.T @ eT          (s1, 64 outputs per batch tile)
    psB = (-s)*ones.T @ (e^2)T (-s*sum e^2; +s*L added as a constant)
    p2  = Square(psA/L) + s*L + psB
The host ships eT = (2x-1) pre-tiled/transposed in bf16 (pure affine fold
into the layout/dtype transform); all reductions, squares and the
nonlinear q=p(1-p) chains stay on device.

Sharding: pure data parallel over batch, 8 cores x 8192 rows, features on
SBUF partitions, batch on the free dim.  All weight prep (binarize,
column sums -> biases, replication) is host-side.  Output is outT
[4, 8192] per core, un-transposed on the host.

Per pair of 512-batch tiles, instruction-interleaved so the two tiles'
matmuls run CONCURRENTLY in different PE column groups (tile_position
(0,0)/(0,64)); e^2 is split DVE (chunks 0..3, tensor_tensor mult, 2x
bf16 mode) / ACT (chunks 4..5, Square).  Layers 2/3 stream p and p^2
(computed by one DVE mult) through +/-s ones matmuls instead of
materializing q = p-p^2.  A 3-stage software pipeline (head / C+combine
one pair later / quad combine one further iteration later) keeps every
op's dependencies at least one pipeline stage old.  Output DMAs and
weight loads ride the gpsimd queue; x rides sync.
"""

import sys

if "/opt/trn_rl_repo" not in sys.path:
    sys.path.insert(0, "/opt/trn_rl_repo")

import ml_dtypes
import numpy as np

BF16 = ml_dtypes.bfloat16

B = 65536
IN_DIM = 768
NCORES = 8
BC = B // NCORES            # 8192 rows per core
TILE = 512                  # batch tile (max fp32 PSUM free dim)
NT = BC // TILE             # 16 tiles per core
NPAIR = NT // 2
NCHUNK = IN_DIM // 128      # 6 feature chunks for layer 1
L1, O1 = 768, 64
L2, O2 = 64, 32
L3, O3 = 32, 4
DVE_SPLIT = 1792            # e^2 cols 0..1791 on DVE (3.5 chunks); rest ACT
ACT_SPLIT = NCHUNK * TILE - DVE_SPLIT
XBUFS = 5                   # x pair-tile prefetch depth (pairs)
# B-matmul segments: (source, src_lo, src_hi, out_lo, out_hi); sources are
# the DVE e^2 tile (chunk-major cols 0..1791) and ACT e^2 tile (1792..3071).
B_SEGS = [
    ("d", 0, 512, 0, 512),
    ("d", 512, 1024, 0, 512),
    ("d", 1024, 1536, 0, 512),
    ("d", 1536, 1792, 0, 256),
    ("a", 0, 256, 256, 512),
    ("a", 256, 768, 0, 512),
    ("a", 768, 1280, 0, 512),
]
# C-term scale: the ones lhsT holds -s in bf16, so use the bf16-rounded
# value as the effective scale for the +s*L1 constant too.
S1_EFF = float(np.float32(BF16(1.0 / (L1 * L1))))
K1 = S1_EFF * L1            # +s*L1 constant  (C = L - sum e^2)
S2 = 4.0 / (L2 * L2)        # 2^-10, exact in bf16
S3 = 4.0 / (L3 * L3)        # 2^-8, exact in bf16

_CACHE = {}


def _build(reps=1, mm_order="interleaved"):
    import contextlib

    import concourse.bacc as bacc
    import concourse.mybir as mybir
    import concourse.tile as tile

    f32 = mybir.dt.float32
    bf16 = mybir.dt.bfloat16
    AOP = mybir.AluOpType
    Square = mybir.ActivationFunctionType.Square

    nc = bacc.Bacc(None, target_bir_lowering=False)

    xt = nc.declare_dram_parameter("xt", [NPAIR, 128, 2, NCHUNK * TILE],
                                   bf16, isOutput=False)
    w1d = nc.declare_dram_parameter("w1d", [128, NCHUNK, O1], bf16,
                                    isOutput=False)
    w2d = nc.declare_dram_parameter("w2d", [128, O2], bf16, isOutput=False)
    w3d = nc.declare_dram_parameter("w3d", [128, L3], bf16, isOutput=False)
    b2d = nc.declare_dram_parameter("b2d", [128, 1], f32, isOutput=False)
    b3d = nc.declare_dram_parameter("b3d", [128, 1], f32, isOutput=False)
    outt = nc.declare_dram_parameter("outt", [O3, BC], f32, isOutput=True)

    NQUAD = NPAIR // 2

    with tile.TileContext(nc) as tc:
        with (
            tc.tile_pool(name="const", bufs=1) as cpool,
            tc.tile_pool(name="xp", bufs=XBUFS) as xpool,
            tc.tile_pool(name="sed", bufs=3) as sedpool,
            tc.tile_pool(name="sea", bufs=3) as seapool,
            tc.tile_pool(name="sb", bufs=2) as spool,
            tc.tile_pool(name="psA", bufs=2, space="PSUM") as pA,
            tc.tile_pool(name="psB", bufs=2, space="PSUM") as pB,
            tc.tile_pool(name="psS", bufs=1, space="PSUM") as pS,
        ):
            # ------------- constants (host-precomputed weights) -------------
            lhsT1 = cpool.tile([128, NCHUNK, O1], bf16)
            nc.gpsimd.dma_start(out=lhsT1, in_=w1d[:])
            lhsT2 = cpool.tile([128, O2], bf16)
            nc.gpsimd.dma_start(out=lhsT2, in_=w2d[:])
            lhsT3 = cpool.tile([128, L3], bf16)
            nc.gpsimd.dma_start(out=lhsT3, in_=w3d[:])
            bias2 = cpool.tile([128, 1], f32)
            nc.gpsimd.dma_start(out=bias2, in_=b2d[:])
            bias3 = cpool.tile([128, 1], f32)
            nc.gpsimd.dma_start(out=bias3, in_=b3d[:])

            onesN1 = cpool.tile([128, O1], bf16)
            nc.vector.memset(onesN1, -S1_EFF)
            onesP2 = cpool.tile([128, O2], bf16)
            nc.vector.memset(onesP2, S2)
            onesN2 = cpool.tile([128, O2], bf16)
            nc.vector.memset(onesN2, -S2)
            onesP3 = cpool.tile([128, L3], bf16)
            nc.vector.memset(onesP3, S3)
            onesN3 = cpool.tile([128, L3], bf16)
            nc.vector.memset(onesN3, -S3)
            zbias = cpool.tile([128, 1], f32)
            nc.vector.memset(zbias, 0.0)

            # PE warmup: ~4.5us of dummy matmuls while the first x pair is
            # still in flight, so the HAM clock-gate reaches 2.4 GHz before
            # real matmuls start (otherwise the first ~3.4us run at 1.2 GHz).
            wtile = cpool.tile([128, TILE], bf16)
            nc.vector.memset(wtile, 0.0)
            psW = pS.tile([128, TILE], f32, tag="A3", name="warm")
            for w in range(20):
                nc.tensor.matmul(
                    psW[0:O1, :], wtile[:, 0:O1], wtile,
                    tile_position=(0, 0),
                )

            # ------------- pipeline stages -------------
            state = {}   # pair -> (se_d, se_a, psA1, psB1)
            cstate = {}  # pair -> (p2p, sq2p) ; quad -> psum/combine tiles

            def head(pr):
                """DMA + s1-matmuls + e^2 for pair pr."""
                x_p = xpool.tile([128, 2, NCHUNK * TILE], bf16, tag="x",
                                 name=f"x_{pr}")
                nc.sync.dma_start(out=x_p, in_=xt[pr])
                psA1 = pA.tile([128, TILE], f32, tag="A1", name=f"psA1_{pr}")
                psB1 = pB.tile([128, TILE], f32, tag="B1", name=f"psB1_{pr}")
                # interleaved s1 matmuls: the two tiles alternate column
                # groups every instruction -> concurrent in the PE array
                if mm_order == "interleaved":
                    a_iter = [(c, k) for c in range(NCHUNK) for k in range(2)]
                else:
                    a_iter = [(c, k) for k in range(2) for c in range(NCHUNK)]
                for c, k in a_iter:
                    nc.tensor.matmul(
                        psA1[O1 * k : O1 * (k + 1), :],
                        lhsT1[:, c, :],
                        x_p[:, k, c * TILE : (c + 1) * TILE],
                        start=(c == 0),
                        stop=(c == NCHUNK - 1),
                        tile_position=(0, O1 * k),
                    )
                # e^2: per-tile 2D ops (3D APs measure ~25% slower)
                se_d = sedpool.tile([128, 2, DVE_SPLIT], bf16, tag="sed",
                                    name=f"sed_{pr}")
                se_a = seapool.tile([128, 2, ACT_SPLIT], bf16, tag="sea",
                                    name=f"sea_{pr}")
                for k in range(2):
                    nc.vector.tensor_mul(
                        se_d[:, k, :],
                        x_p[:, k, 0:DVE_SPLIT],
                        x_p[:, k, 0:DVE_SPLIT],
                    )
                for k in range(2):
                    nc.scalar.activation(
                        se_a[:, k, :], x_p[:, k, DVE_SPLIT:], Square,
                        bias=zbias, scale=1.0,
                    )
                state[pr] = (se_d, se_a, psA1, psB1)

            def tailB(pr):
                """C-term matmuls + layer-1 combine + layer-2 prep, pair pr.

                Emits: PE B-MMs, ACT t2p, DVE p2p + sq2p.  (The layer-2
                matmuls are emitted separately in tailL2 so the PE can run
                the quad's layer-3 matmuls first.)
                """
                se_d, se_a, psA1, psB1 = state.pop(pr)
                # C-term: psB1 = (-s)*ones.T @ e^2, interleaved tiles
                nseg = len(B_SEGS)
                if mm_order == "interleaved":
                    b_iter = [(s, k) for s in range(nseg) for k in range(2)]
                else:
                    b_iter = [(s, k) for k in range(2) for s in range(nseg)]
                for s, k in b_iter:
                    which, lo, hi, olo, ohi = B_SEGS[s]
                    tile_src = se_d if which == "d" else se_a
                    nc.tensor.matmul(
                        psB1[O1 * k : O1 * k + O1, olo:ohi],
                        onesN1,
                        tile_src[:, k, lo:hi],
                        start=(s == 0),
                        stop=(s == nseg - 1),
                        tile_position=(0, O1 * k),
                    )
                t2p = spool.tile([128, TILE], f32, tag="t2", name=f"t2_{pr}")
                nc.scalar.activation(
                    t2p, psA1, Square, bias=zbias, scale=1.0 / L1
                )
                p2p = spool.tile([128, TILE], bf16, tag="p2", name=f"p2_{pr}")
                nc.vector.scalar_tensor_tensor(
                    p2p, t2p, K1, psB1, AOP.add, AOP.add
                )
                sq2p = spool.tile([128, TILE], bf16, tag="sq2",
                                  name=f"sq2_{pr}")
                nc.vector.tensor_mul(sq2p, p2p, p2p)
                cstate[pr] = (p2p, sq2p)

            def tailL2(pr):
                """Layer-2 matmuls for pair pr (p2 and p2^2 streams)."""
                p2p, sq2p = cstate.pop(pr)
                qj = (2 * pr + 1) % 4
                if qj == 1:
                    psA2 = pA.tile([128, TILE], f32, tag="A2", bufs=1,
                                   name=f"psA2_{pr}")
                    psB2 = pB.tile([128, TILE], f32, tag="B2", bufs=1,
                                   name=f"psB2_{pr}")
                    cstate[("q", pr // 2)] = (psA2, psB2)
                else:
                    psA2, psB2 = cstate[("q", pr // 2)]
                # each tile of the pair produces its own 32-row block:
                # psA2 gets one K=64 matmul per block; psB2 accumulates the
                # p2-stream and the (-)p2^2-stream per block.  half-
                # interleaved so the two blocks run concurrently.
                halves = ((0, qj - 1), (1, qj))
                for half, tq in halves:
                    hs = slice(O1 * half, O1 * (half + 1))
                    nc.tensor.matmul(
                        psA2[O2 * tq : O2 * (tq + 1), :],
                        lhsT2[hs, :],
                        p2p[hs, :],
                        tile_position=(O1 * half, O2 * tq),
                    )
                for half, tq in halves:
                    hs = slice(O1 * half, O1 * (half + 1))
                    nc.tensor.matmul(
                        psB2[O2 * tq : O2 * (tq + 1), :],
                        onesP2[hs, :],
                        p2p[hs, :],
                        start=True,
                        stop=False,
                        tile_position=(O1 * half, O2 * tq),
                    )
                for half, tq in halves:
                    hs = slice(O1 * half, O1 * (half + 1))
                    nc.tensor.matmul(
                        psB2[O2 * tq : O2 * (tq + 1), :],
                        onesN2[hs, :],
                        sq2p[hs, :],
                        start=False,
                        stop=True,
                        tile_position=(O1 * half, O2 * tq),
                    )

            def quad_pre(Q):
                """Layer-2 combine, ACT part (earliest, old deps)."""
                psA2, psB2 = cstate.pop(("q", Q))
                tq2 = spool.tile([128, TILE], f32, tag="tq2", name=f"tq2_{Q}")
                nc.scalar.activation(
                    tq2, psA2, Square, bias=bias2, scale=2.0 / L2
                )
                cstate[("qc", Q)] = (psB2, tq2)

            def quad_dve(Q):
                """Layer-2 combine, DVE part -> p3, p3^2."""
                psB2, tq2 = cstate.pop(("qc", Q))
                p3q = spool.tile([128, TILE], bf16, tag="p3", name=f"p3_{Q}")
                nc.vector.scalar_tensor_tensor(
                    p3q, tq2, 0.0, psB2, AOP.add, AOP.add
                )
                sq3q = spool.tile([128, TILE], bf16, tag="sq3",
                                  name=f"sq3_{Q}")
                nc.vector.tensor_mul(sq3q, p3q, p3q)
                cstate[("q3", Q)] = (p3q, sq3q)

            def quad_l3(Q):
                """Layer-3 matmuls + combine + output DMA for quad Q."""
                p3q, sq3q = cstate.pop(("q3", Q))
                psA3 = pS.tile([128, TILE], f32, tag="A3", name=f"psA3_{Q}")
                psB3 = pS.tile([128, TILE], f32, tag="B3", name=f"psB3_{Q}")
                # j-interleaved: the 4 diagonal 32x32 positions run
                # concurrently in the PE array (3 slot-times per quad)
                for j in range(4):
                    js = slice(32 * j, 32 * j + L3)
                    nc.tensor.matmul(
                        psA3[32 * j : 32 * j + L3, :],
                        lhsT3[js, :],
                        p3q[js, :],
                        tile_position=(32 * j, 32 * j),
                    )
                for j in range(4):
                    js = slice(32 * j, 32 * j + L3)
                    nc.tensor.matmul(
                        psB3[32 * j : 32 * j + L3, :],
                        onesP3[js, :],
                        p3q[js, :],
                        start=True,
                        stop=False,
                        tile_position=(32 * j, 32 * j),
                    )
                for j in range(4):
                    js = slice(32 * j, 32 * j + L3)
                    nc.tensor.matmul(
                        psB3[32 * j : 32 * j + L3, :],
                        onesN3[js, :],
                        sq3q[js, :],
                        start=False,
                        stop=True,
                        tile_position=(32 * j, 32 * j),
                    )
                t3q = spool.tile([128, TILE], f32, tag="t3", name=f"t3_{Q}")
                nc.scalar.activation(
                    t3q, psA3, Square, bias=bias3, scale=2.0 / L3
                )
                outq = spool.tile([128, TILE], f32, tag="outq",
                                  name=f"outq_{Q}")
                nc.vector.scalar_tensor_tensor(
                    outq, t3q, 0.0, psB3, AOP.add, AOP.add
                )
                for j in range(4):
                    tt = 4 * Q + j
                    nc.gpsimd.dma_start(
                        out=outt[:, tt * TILE : (tt + 1) * TILE],
                        in_=outq[32 * j : 32 * j + O3, :],
                    )

            loop_cm = (
                tc.For_i(0, reps, 1) if reps > 1 else contextlib.nullcontext()
            )
            with loop_cm:
                # Steady state: head(it) | tailB(it-1) | tailL2(it-2) |
                # quad(Q) at it=2Q+4.  For the last two pairs the tail
                # stages are pulled forward: the PE would otherwise idle
                # waiting for the final DMA, and the drain chain after the
                # last pair shortens by a full iteration.
                for it in range(NPAIR - 1):
                    Q = (it - 4) // 2
                    do_quad = it >= 4 and (it - 4) % 2 == 0 and Q < NQUAD
                    if do_quad:
                        quad_pre(Q)
                    head(it)
                    if do_quad:
                        quad_dve(Q)
                    if it >= 1:
                        tailB(it - 1)
                    if do_quad:
                        quad_l3(Q)
                    if it >= 2:
                        tailL2(it - 2)
                # it = NPAIR-1: tail work first (PE is waiting on the last
                # DMA anyway), then the last head, then pair NPAIR-2's
                # combine chain pulled in early.
                tailB(NPAIR - 2)
                head(NPAIR - 1)
                tailL2(NPAIR - 3)
                # quad(NQUAD-2) must read its psA2/psB2 slot before
                # tailL2(NPAIR-2) reallocates it
                quad_pre(NQUAD - 2)
                quad_dve(NQUAD - 2)
                tailL2(NPAIR - 2)
                # drain: last pair's chain + the last two quads
                tailB(NPAIR - 1)
                quad_l3(NQUAD - 2)
                tailL2(NPAIR - 1)
                quad_pre(NQUAD - 1)
                quad_dve(NQUAD - 1)
                quad_l3(NQUAD - 1)

    nc.compile()
    return nc


def _get_nc(reps=1, mm_order="interleaved"):
    key = ("nc", reps, mm_order)
    if key not in _CACHE:
        _CACHE[key] = _build(reps, mm_order)
    return _CACHE[key]


def _make_in_maps(x, w1, w2, w3):
    x = np.asarray(x, dtype=np.float32)
    w1 = np.asarray(w1, dtype=np.float32)
    w2 = np.asarray(w2, dtype=np.float32)
    w3 = np.asarray(w3, dtype=np.float32)

    # binarized weights and their column sums (all tiny -> host)
    wb1 = np.where(w1 >= 0, 1.0, -1.0).astype(np.float32)   # [64, 768]
    wb2 = np.where(w2 >= 0, 1.0, -1.0).astype(np.float32)   # [32, 64]
    wb3 = np.where(w3 >= 0, 1.0, -1.0).astype(np.float32)   # [4, 32]

    # lhsT1[p, c, o] = wb1[o, c*128+p]
    w1d = np.ascontiguousarray(
        wb1.T.reshape(NCHUNK, 128, O1).transpose(1, 0, 2)
    ).astype(BF16)
    # lhsT2: [128, 32], rows 0..63 = wb2.T, rows 64..127 = copy
    w2d = np.ascontiguousarray(np.tile(wb2.T, (2, 1))).astype(BF16)
    # lhsT3: [128, 32], wb3.T replicated 4x in cols 0..3, zeros in 4..31
    # (zero columns keep the padded psum rows written so every PSUM read
    #  touches initialized memory)
    w3d = np.zeros((128, L3), np.float32)
    w3d[:, :O3] = np.tile(wb3.T, (4, 1))
    w3d = np.ascontiguousarray(w3d).astype(BF16)

    c2 = wb2.sum(axis=1)   # [32]
    c3 = wb3.sum(axis=1)   # [4]
    b2d = np.ascontiguousarray(
        np.tile(-c2 / L2, 4).reshape(128, 1)
    ).astype(np.float32)
    b3 = np.zeros((4, 32), np.float32)
    b3[:, :O3] = -c3 / L3
    b3d = np.ascontiguousarray(b3.reshape(128, 1))

    xs = x.reshape(NCORES, NT, TILE, NCHUNK, 128)
    # eT = 2x-1, [core][pair, partition(f%128), tile-in-pair,
    #             chunk(f//128)*TILE + batch-in-tile], bf16
    et = (xs.transpose(0, 1, 4, 3, 2) * np.float32(2.0) - np.float32(1.0))
    etiled = np.ascontiguousarray(et.astype(BF16)).reshape(
        NCORES, NPAIR, 2, 128, NCHUNK * TILE
    ).transpose(0, 1, 3, 2, 4)
    etiled = np.ascontiguousarray(etiled)
    return [
        {
            "xt": etiled[i],
            "w1d": w1d,
            "w2d": w2d,
            "w3d": w3d,
            "b2d": b2d,
            "b3d": b3d,
        }
        for i in range(NCORES)
    ]


def kernel(x, w1, w2, w3):
    from concourse.bass_utils import run_bass_kernel_spmd

    nc = _get_nc()
    in_maps = _make_in_maps(x, w1, w2, w3)
    res = run_bass_kernel_spmd(nc, in_maps, core_ids=list(range(NCORES)))
    return np.concatenate(
        [res.results[i]["outt"].T for i in range(NCORES)], axis=0
    ).astype(np.float32)


def bench(x, w1, w2, w3, iters=20, reps=1, cores=NCORES):
    """Time device execution with a persistent jit and device-resident
    inputs (excludes host<->device transfer and compile).  Returns
    (output, per_call_seconds_list).  NOTE: per-call wall time under axon
    is dominated by a fixed ~80ms relay dispatch latency; use the NTFF
    profile (run_bass_kernel_spmd(trace=True)) for true HW exec time."""
    import time

    import jax
    from jax.sharding import Mesh, NamedSharding, PartitionSpec
    from jax.experimental.shard_map import shard_map

    import concourse.mybir as mybir
    from concourse import bass2jax
    from concourse.bass2jax import _bass_exec_p, install_neuronx_cc_hook

    nc = _get_nc(reps)
    install_neuronx_cc_hook()
    in_maps = _make_in_maps(x, w1, w2, w3)

    partition_name = (
        nc.partition_id_tensor.name if nc.partition_id_tensor else None
    )
    in_names, out_names, out_avals, zero_outs = [], [], [], []
    for alloc in nc.m.functions[0].allocations:
        if not isinstance(alloc, mybir.MemoryLocationSet):
            continue
        name = alloc.memorylocations[0].name
        if alloc.kind == "ExternalInput":
            if name != partition_name:
                in_names.append(name)
        elif alloc.kind == "ExternalOutput":
            out_names.append(name)
            shape = tuple(alloc.tensor_shape)
            dtype = mybir.dt.np(alloc.dtype)
            out_avals.append(jax.core.ShapedArray(shape, dtype))
            zero_outs.append(np.zeros(shape, dtype))
    n_params = len(in_names)
    in_names = in_names + out_names
    if partition_name is not None:
        in_names = in_names + [partition_name]

    def _body(*args):
        operands = list(args)
        if partition_name is not None:
            operands.append(bass2jax.partition_id_tensor())
        outs = _bass_exec_p.bind(
            *operands,
            out_avals=tuple(out_avals),
            in_names=tuple(in_names),
            out_names=tuple(out_names),
            lowering_input_output_aliases=(),
            sim_require_finite=True,
            sim_require_nnan=True,
            nc=nc,
        )
        return tuple(outs)

    devices = jax.devices()[:cores]
    mesh = Mesh(np.asarray(devices), ("core",))
    in_specs = (PartitionSpec("core"),) * (n_params + len(out_names))
    out_specs = (PartitionSpec("core"),) * len(out_names)
    fn = jax.jit(
        shard_map(_body, mesh=mesh, in_specs=in_specs, out_specs=out_specs,
                  check_rep=False),
        keep_unused=True,
    )
    sh = NamedSharding(mesh, PartitionSpec("core"))
    dev_in = [
        jax.device_put(
            np.concatenate([in_maps[c][nm] for c in range(cores)], axis=0), sh
        )
        for nm in in_names[:n_params]
    ]
    dev_zero = [
        jax.device_put(
            np.zeros((cores * z.shape[0], *z.shape[1:]), z.dtype), sh
        )
        for z in zero_outs
    ]
    out = fn(*dev_in, *dev_zero)
    jax.block_until_ready(out)
    times = []
    for _ in range(iters):
        t0 = time.perf_counter()
        out = fn(*dev_in, *dev_zero)
        jax.block_until_ready(out)
        times.append(time.perf_counter() - t0)
    out_np = np.asarray(out[0]).reshape(cores, *out_avals[0].shape)
    result = np.concatenate([out_np[c].T for c in range(cores)], axis=0)
    return result.astype(np.float32), times
